# revision 8
# baseline (speedup 1.0000x reference)
"""HQQ grouped (per-expert) int4-dequant GEMM on 8 trn2 NeuronCores.

Math per expert e (group g = k // group_size):
    W_e[k, n] = (q_e[k, n] - 8) * scale_e[g, n] + zero_e[g, n]
    out[rows_e] = x[rows_e] @ W_e          (rows_e contiguous, expert-sorted)

Sharding: 16 units = (expert, out-half).  Each core gets two units (slot A,
slot B) chosen so per-core token counts balance.  Host repacks int4 codes to
int8 and transposes x to bf16; the device casts q to bf16 (gpsimd cast-DMA),
multiplies by a partition-replicated scale tile (DVE tensor_tensor, 2x bf16)
and accumulates x^T.T @ (q*scale) over 16 K-tiles plus one extra K-tile that
applies the folded zero-point term  xs @ (zero - 8*scale)  where xs are the
per-group sums of x.
"""

import math
import os

import ml_dtypes
import numpy as np


def _ensure_ntff_hook():
    """The agent image's `antenv` lacks `axon_hooks`, so boot() skipped
    registering the NTFF profiling hook and trace=True would degrade to a
    no-profile run.  Recreate the module + register the ctypes hook."""
    import sys
    import types

    if "antenv.axon_hooks" in sys.modules:
        return
    try:
        import antenv  # noqa: F401

        mod = types.ModuleType("antenv.axon_hooks")
        mod._hook = None

        def set_axon_ntff_profile_hook(h):
            mod._hook = h

        def get_axon_ntff_profile_hook():
            return mod._hook

        mod.set_axon_ntff_profile_hook = set_axon_ntff_profile_hook
        mod.get_axon_ntff_profile_hook = get_axon_ntff_profile_hook
        sys.modules["antenv.axon_hooks"] = mod

        from trn_agent_boot.trn_boot import _ntff_profile_via_ctypes

        hook = _ntff_profile_via_ctypes("/opt/axon/libaxon_pjrt.so")
        if hook is not None:
            set_axon_ntff_profile_hook(hook)
    except Exception:
        pass


_ensure_ntff_hook()

E, T, IN, OUT = 8, 2048, 2048, 2048
P = 128
NCORES = 8
NHALF = OUT // 2  # 1024
MM_N = 512  # one PSUM bank
BF16 = ml_dtypes.bfloat16

_PROGRAM_CACHE = {}
LAST_RESULT = None


def _build_program(a_cap, b_cap, gs):
    """Build + compile the SPMD Bass program for slot capacities (in 128-token
    tiles) a_cap/b_cap and quant group size gs."""
    import concourse.bacc as bacc
    import concourse.mybir as mybir
    import concourse.tile as tile
    from contextlib import ExitStack

    bf16 = mybir.dt.bfloat16
    f32 = mybir.dt.float32
    i8 = mybir.dt.int8

    G = IN // gs       # quant groups (32)
    KT = IN // P       # 16 k-tiles
    KT1 = KT + 1       # + zero-point k-tile
    RPT = P // gs      # groups spanned by one 128-row k-tile (2)
    CH = 4             # q k-tiles per cast-DMA chunk

    nc = bacc.Bacc(
        "TRN2",
        target_bir_lowering=False,
        debug=False,
        enable_asserts=True,
        num_devices=NCORES,
    )

    slots = []
    for name, cap in (("a", a_cap), ("b", b_cap)):
        capT = cap * P
        slots.append(
            dict(
                name=name,
                cap=cap,
                capT=capT,
                # partition-major packed: [P, KT, NHALF] int8, 16KB runs
                q=nc.dram_tensor(f"q{name}", [P, KT, NHALF], i8, kind="ExternalInput").ap(),
                s=nc.dram_tensor(f"s{name}", [G, NHALF], bf16, kind="ExternalInput").ap(),
                z=nc.dram_tensor(f"z{name}", [P, NHALF], bf16, kind="ExternalInput").ap(),
                # partition-major packed x (incl. group-sum rows at kt=KT)
                xt=nc.dram_tensor(f"x{name}", [P, KT1, capT], bf16, kind="ExternalInput").ap(),
                y=nc.dram_tensor(f"y{name}", [capT, OUT // 2], f32, kind="ExternalOutput").ap(),
            )
        )

    with tile.TileContext(nc) as tc, ExitStack() as ctx:
        xpool = ctx.enter_context(tc.tile_pool(name="x", bufs=1))
        wpool = ctx.enter_context(tc.tile_pool(name="w", bufs=2))
        qpool = ctx.enter_context(tc.tile_pool(name="q", bufs=2))
        rpool = ctx.enter_context(tc.tile_pool(name="rep", bufs=1))
        opool = ctx.enter_context(tc.tile_pool(name="o", bufs=2))
        pspool = ctx.enter_context(tc.tile_pool(name="ps", bufs=4, space="PSUM"))

        for sl in slots:
            cap, capT = sl["cap"], sl["capT"]
            nm = sl["name"]

            x_sb = xpool.tile([P, KT1, capT], bf16, tag=f"x{nm}")
            nc.sync.dma_start(x_sb[:], sl["xt"])

            w_sb = wpool.tile([P, KT1, NHALF], bf16, tag="w")
            # zero-point rows go straight into the last k-tile of W
            nc.sync.dma_start(w_sb[:, KT, :], sl["z"])

            # replicated scales: srep[p, kt, n] = scale[RPT*kt + p//gs, n].
            # Two-level fanout-of-8 (depth 2) instead of a 6-deep doubling
            # chain: dependent DMA hops cost ~5us each in queue latency.
            srep = rpool.tile([P, KT, NHALF], bf16, tag=f"srep{nm}")
            nc.sync.dma_start(
                srep.rearrange("(j r) kt n -> j r kt n", j=RPT)[:, 0:1],
                sl["s"].rearrange("(kt j) n -> j kt n", j=RPT)[:, None],
            )
            assert gs == 64, "fanout below assumes 64x replication"
            for h in range(RPT):
                base = h * gs
                eng = nc.scalar if h == 0 else nc.sync
                for i in range(1, 8):
                    eng.dma_start(srep[base + i : base + i + 1], srep[base : base + 1])
                for i in range(1, 8):
                    eng.dma_start(
                        srep[base + 8 * i : base + 8 * (i + 1)], srep[base : base + 8]
                    )

            for c in range(KT // CH):
                qb = qpool.tile([P, CH, NHALF], bf16, tag="qb")
                nc.gpsimd.dma_start(qb[:], sl["q"][:, c * CH : (c + 1) * CH, :])
                nc.vector.tensor_tensor(
                    w_sb[:, c * CH : (c + 1) * CH, :],
                    qb[:],
                    srep[:, c * CH : (c + 1) * CH, :],
                    mybir.AluOpType.mult,
                )

            for tt in range(cap):
                o_sb = opool.tile([P, NHALF], f32, tag="o")
                for nb in range(NHALF // MM_N):
                    ps = pspool.tile([P, MM_N], f32, tag="ps")
                    for kt in range(KT1):
                        nc.tensor.matmul(
                            ps[:],
                            x_sb[:, kt, tt * P : (tt + 1) * P],
                            w_sb[:, kt, nb * MM_N : (nb + 1) * MM_N],
                            start=(kt == 0),
                            stop=(kt == KT1 - 1),
                        )
                    nc.any.tensor_copy(
                        out=o_sb[:, nb * MM_N : (nb + 1) * MM_N], in_=ps[:]
                    )
                nc.sync.dma_start(sl["y"][tt * P : (tt + 1) * P, :], o_sb[:])

    nc.compile()
    return nc


def _plan(tokens_per_expert):
    """Assign the 16 (expert, half) units to 8 cores x 2 slots."""
    tpe = np.asarray(tokens_per_expert).astype(np.int64)
    units = []
    for e in range(E):
        tt = int(math.ceil(tpe[e] / P))
        for h in range(2):
            units.append((tt, e, h))
    units.sort(key=lambda u: -u[0])
    a_units, b_units = units[:NCORES], units[NCORES:]
    # pair biggest A with smallest B for mild DMA smoothing
    b_units = b_units[::-1]
    a_cap = max(1, max(u[0] for u in a_units))
    b_cap = max(1, max(u[0] for u in b_units))
    return a_units, b_units, a_cap, b_cap


def kernel(x, qweight, scales_and_zeros, tokens_per_expert, group_size, **_):
    global LAST_RESULT
    from concourse.bass_utils import run_bass_kernel_spmd

    gs = int(group_size)
    G = IN // gs

    x = np.asarray(x, dtype=np.float32)
    qweight = np.asarray(qweight)
    snz = np.asarray(scales_and_zeros, dtype=np.float32)
    tpe = np.asarray(tokens_per_expert).astype(np.int64)
    bounds = np.concatenate([[0], np.cumsum(tpe)]).astype(np.int64)

    a_units, b_units, a_cap, b_cap = _plan(tpe)
    key = (a_cap, b_cap, gs)
    if key not in _PROGRAM_CACHE:
        _PROGRAM_CACHE[key] = _build_program(a_cap, b_cap, gs)
    nc = _PROGRAM_CACHE[key]

    # host-side layout prep (value-preserving repack/transpose/cast only)
    KT = IN // P
    # x packed partition-major: xpm[p, kt, t] = x[t, kt*128+p], bf16
    xpm = np.ascontiguousarray(
        x.T.reshape(KT, P, T).transpose(1, 0, 2)
    ).astype(BF16)                                                   # [P, KT, T]
    xs_all = np.ascontiguousarray(
        x.reshape(T, G, gs).sum(axis=2, dtype=np.float32).T
    ).astype(BF16)                                                   # [G, T]
    # q packed partition-major: qpm[e, p, kt, n] = q[e, kt*128+p, n], int8
    qpm = np.ascontiguousarray(
        qweight.astype(np.int8).reshape(E, KT, P, OUT).transpose(0, 2, 1, 3)
    )                                                                # [E, P, KT, OUT]
    sc = snz[..., 0]                                                 # [E, G, OUT]
    zp = (snz[..., 1] - 8.0 * sc).astype(BF16)                       # zero' = zero-8*scale
    sc16 = sc.astype(BF16)

    in_maps = []
    for c in range(NCORES):
        m = {}
        for slot, cap, (tt, e, h) in (("a", a_cap, a_units[c]), ("b", b_cap, b_units[c])):
            capT = cap * P
            r0, r1 = int(bounds[e]), int(bounds[e + 1])
            n0, n1 = h * NHALF, (h + 1) * NHALF
            xa = np.zeros([P, KT + 1, capT], BF16)
            xa[:, :KT, : r1 - r0] = xpm[:, :, r0:r1]
            xa[:G, KT, : r1 - r0] = xs_all[:, r0:r1]
            za = np.zeros([P, NHALF], BF16)
            za[:G] = zp[e, :, n0:n1]
            m[f"x{slot}"] = xa
            m[f"q{slot}"] = np.ascontiguousarray(qpm[e, :, :, n0:n1])
            m[f"s{slot}"] = np.ascontiguousarray(sc16[e, :, n0:n1])
            m[f"z{slot}"] = za
        in_maps.append(m)

    res = run_bass_kernel_spmd(nc, in_maps, list(range(NCORES)))
    LAST_RESULT = res

    out = np.zeros([T, OUT], np.float32)
    for c in range(NCORES):
        for slot, (tt, e, h) in (("a", a_units[c]), ("b", b_units[c])):
            r0, r1 = int(bounds[e]), int(bounds[e + 1])
            out[r0:r1, h * NHALF : (h + 1) * NHALF] = res.results[c][f"y{slot}"][
                : r1 - r0
            ]
    return out


# revision 13
# speedup vs baseline: 1.4736x; 1.4736x over previous
"""HQQ grouped (per-expert) int4-dequant GEMM on 8 trn2 NeuronCores.

Math per expert e (group g = k // group_size):
    W_e[k, n] = (q_e[k, n] - 8) * scale_e[g, n] + zero_e[g, n]
    out[rows_e] = x[rows_e] @ W_e          (rows_e contiguous, expert-sorted)

Sharding: 16 units = (expert, out-half).  Each core gets two units (slot A,
slot B) chosen so per-core token counts balance.  Host repacks int4 codes to
int8 and transposes x to bf16; the device casts q to bf16 (gpsimd cast-DMA),
multiplies by a partition-replicated scale tile (DVE tensor_tensor, 2x bf16)
and accumulates x^T.T @ (q*scale) over 16 K-tiles plus one extra K-tile that
applies the folded zero-point term  xs @ (zero - 8*scale)  where xs are the
per-group sums of x.
"""

import math
import os

import ml_dtypes
import numpy as np


def _ensure_ntff_hook():
    """The agent image's `antenv` lacks `axon_hooks`, so boot() skipped
    registering the NTFF profiling hook and trace=True would degrade to a
    no-profile run.  Recreate the module + register the ctypes hook."""
    import sys
    import types

    if "antenv.axon_hooks" in sys.modules:
        return
    try:
        import antenv  # noqa: F401

        mod = types.ModuleType("antenv.axon_hooks")
        mod._hook = None

        def set_axon_ntff_profile_hook(h):
            mod._hook = h

        def get_axon_ntff_profile_hook():
            return mod._hook

        mod.set_axon_ntff_profile_hook = set_axon_ntff_profile_hook
        mod.get_axon_ntff_profile_hook = get_axon_ntff_profile_hook
        sys.modules["antenv.axon_hooks"] = mod

        from trn_agent_boot.trn_boot import _ntff_profile_via_ctypes

        hook = _ntff_profile_via_ctypes("/opt/axon/libaxon_pjrt.so")
        if hook is not None:
            set_axon_ntff_profile_hook(hook)
    except Exception:
        pass


_ensure_ntff_hook()

E, T, IN, OUT = 8, 2048, 2048, 2048
P = 128
NCORES = 8
NHALF = OUT // 2  # 1024
MM_N = 512  # one PSUM bank
BF16 = ml_dtypes.bfloat16

_PROGRAM_CACHE = {}
LAST_RESULT = None


def _build_program(a_cap, b_cap, gs):
    """Build + compile the SPMD Bass program for slot capacities (in 128-token
    tiles) a_cap/b_cap and quant group size gs."""
    import concourse.bacc as bacc
    import concourse.mybir as mybir
    import concourse.tile as tile
    from contextlib import ExitStack

    bf16 = mybir.dt.bfloat16
    f32 = mybir.dt.float32
    i8 = mybir.dt.int8

    G = IN // gs       # quant groups (32)
    KT = IN // P       # 16 k-tiles
    KT1 = KT + 1       # + zero-point k-tile
    RPT = P // gs      # groups spanned by one 128-row k-tile (2)
    CH = 4             # q k-tiles per cast-DMA chunk

    nc = bacc.Bacc(
        "TRN2",
        target_bir_lowering=False,
        debug=False,
        enable_asserts=True,
        num_devices=NCORES,
    )

    slots = []
    for name, cap in (("a", a_cap), ("b", b_cap)):
        capT = cap * P
        slots.append(
            dict(
                name=name,
                cap=cap,
                capT=capT,
                # partition-major packed: [P, KT, NHALF] int8, 16KB runs
                q=nc.dram_tensor(f"q{name}", [P, KT, NHALF], i8, kind="ExternalInput").ap(),
                s=nc.dram_tensor(f"s{name}", [G, NHALF], bf16, kind="ExternalInput").ap(),
                z=nc.dram_tensor(f"z{name}", [P, NHALF], bf16, kind="ExternalInput").ap(),
                # partition-major packed x (incl. group-sum rows at kt=KT)
                xt=nc.dram_tensor(f"x{name}", [P, KT1, capT], bf16, kind="ExternalInput").ap(),
                y=nc.dram_tensor(f"y{name}", [capT, OUT // 2], f32, kind="ExternalOutput").ap(),
            )
        )

    bsel_dram = nc.dram_tensor("bsel", [RPT, P], bf16, kind="ExternalInput").ap()

    with tile.TileContext(nc) as tc, ExitStack() as ctx:
        xpool = ctx.enter_context(tc.tile_pool(name="x", bufs=1))
        wpool = ctx.enter_context(tc.tile_pool(name="w", bufs=2))
        qpool = ctx.enter_context(tc.tile_pool(name="q", bufs=2))
        rpool = ctx.enter_context(tc.tile_pool(name="rep", bufs=4))
        spool = ctx.enter_context(tc.tile_pool(name="sc", bufs=1))
        opool = ctx.enter_context(tc.tile_pool(name="o", bufs=2))
        pspool = ctx.enter_context(tc.tile_pool(name="ps", bufs=4, space="PSUM"))
        psrpool = ctx.enter_context(tc.tile_pool(name="psr", bufs=2, space="PSUM"))

        # selection matrix for PE scale-broadcast: bsel[j, p] = (p//gs == j)
        bsel = spool.tile([RPT, P], bf16, tag="bsel")
        nc.sync.dma_start(bsel[:], bsel_dram)

        for sl in slots:
            cap, capT = sl["cap"], sl["capT"]
            nm = sl["name"]

            x_sb = xpool.tile([P, KT1, capT], bf16, tag=f"x{nm}")
            nc.sync.dma_start(x_sb[:], sl["xt"])

            w_sb = wpool.tile([P, KT1, NHALF], bf16, tag="w")
            # zero-point rows go straight into the last k-tile of W
            nc.sync.dma_start(w_sb[:, KT, :], sl["z"])

            # replicated scales srep[p, kt, n] = scale[RPT*kt + p//gs, n],
            # built on the PE (K=RPT broadcast matmuls) + ScalarE copy-back.
            # This keeps the 64x replication off the DMA fabric entirely.
            s_sb = spool.tile([RPT, KT, NHALF], bf16, tag="s")
            nc.sync.dma_start(s_sb[:], sl["s"].rearrange("(kt j) n -> j kt n", j=RPT))
            for c in range(KT // CH):
                srep = rpool.tile([P, CH, NHALF], bf16, tag="srepch")
                for j in range(CH):
                    for nb in range(NHALF // MM_N):
                        psr = psrpool.tile([P, MM_N], f32, tag="psr")
                        nc.tensor.matmul(
                            psr[:],
                            bsel[:],
                            s_sb[:, c * CH + j, nb * MM_N : (nb + 1) * MM_N],
                            start=True,
                            stop=True,
                        )
                        nc.scalar.copy(
                            out=srep[:, j, nb * MM_N : (nb + 1) * MM_N], in_=psr[:]
                        )
                qb = qpool.tile([P, CH, NHALF], bf16, tag="qb")
                nc.gpsimd.dma_start(qb[:], sl["q"][:, c * CH : (c + 1) * CH, :])
                nc.vector.tensor_tensor(
                    w_sb[:, c * CH : (c + 1) * CH, :],
                    qb[:],
                    srep[:],
                    mybir.AluOpType.mult,
                )

            for tt in range(cap):
                o_sb = opool.tile([P, NHALF], f32, tag="o")
                for nb in range(NHALF // MM_N):
                    ps = pspool.tile([P, MM_N], f32, tag="ps")
                    for kt in range(KT1):
                        nc.tensor.matmul(
                            ps[:],
                            x_sb[:, kt, tt * P : (tt + 1) * P],
                            w_sb[:, kt, nb * MM_N : (nb + 1) * MM_N],
                            start=(kt == 0),
                            stop=(kt == KT1 - 1),
                        )
                    nc.any.tensor_copy(
                        out=o_sb[:, nb * MM_N : (nb + 1) * MM_N], in_=ps[:]
                    )
                nc.sync.dma_start(sl["y"][tt * P : (tt + 1) * P, :], o_sb[:])

    nc.compile()
    return nc


def _plan(tokens_per_expert):
    """Assign the 16 (expert, half) units to 8 cores x 2 slots."""
    tpe = np.asarray(tokens_per_expert).astype(np.int64)
    units = []
    for e in range(E):
        tt = int(math.ceil(tpe[e] / P))
        for h in range(2):
            units.append((tt, e, h))
    units.sort(key=lambda u: -u[0])
    a_units, b_units = units[:NCORES], units[NCORES:]
    # pair biggest A with smallest B for mild DMA smoothing
    b_units = b_units[::-1]
    a_cap = max(1, max(u[0] for u in a_units))
    b_cap = max(1, max(u[0] for u in b_units))
    return a_units, b_units, a_cap, b_cap


def kernel(x, qweight, scales_and_zeros, tokens_per_expert, group_size, **_):
    global LAST_RESULT
    from concourse.bass_utils import run_bass_kernel_spmd

    gs = int(group_size)
    G = IN // gs

    x = np.asarray(x, dtype=np.float32)
    qweight = np.asarray(qweight)
    snz = np.asarray(scales_and_zeros, dtype=np.float32)
    tpe = np.asarray(tokens_per_expert).astype(np.int64)
    bounds = np.concatenate([[0], np.cumsum(tpe)]).astype(np.int64)

    a_units, b_units, a_cap, b_cap = _plan(tpe)
    key = (a_cap, b_cap, gs)
    if key not in _PROGRAM_CACHE:
        _PROGRAM_CACHE[key] = _build_program(a_cap, b_cap, gs)
    nc = _PROGRAM_CACHE[key]

    # host-side layout prep (value-preserving repack/transpose/cast only)
    KT = IN // P
    # x packed partition-major: xpm[p, kt, t] = x[t, kt*128+p], bf16
    xpm = np.ascontiguousarray(
        x.T.reshape(KT, P, T).transpose(1, 0, 2)
    ).astype(BF16)                                                   # [P, KT, T]
    xs_all = np.ascontiguousarray(
        x.reshape(T, G, gs).sum(axis=2, dtype=np.float32).T
    ).astype(BF16)                                                   # [G, T]
    # q packed partition-major: qpm[e, p, kt, n] = q[e, kt*128+p, n], int8
    qpm = np.ascontiguousarray(
        qweight.astype(np.int8).reshape(E, KT, P, OUT).transpose(0, 2, 1, 3)
    )                                                                # [E, P, KT, OUT]
    sc = snz[..., 0]                                                 # [E, G, OUT]
    zp = (snz[..., 1] - 8.0 * sc).astype(BF16)                       # zero' = zero-8*scale
    sc16 = sc.astype(BF16)

    in_maps = []
    for c in range(NCORES):
        m = {}
        for slot, cap, (tt, e, h) in (("a", a_cap, a_units[c]), ("b", b_cap, b_units[c])):
            capT = cap * P
            r0, r1 = int(bounds[e]), int(bounds[e + 1])
            n0, n1 = h * NHALF, (h + 1) * NHALF
            xa = np.zeros([P, KT + 1, capT], BF16)
            xa[:, :KT, : r1 - r0] = xpm[:, :, r0:r1]
            xa[:G, KT, : r1 - r0] = xs_all[:, r0:r1]
            za = np.zeros([P, NHALF], BF16)
            za[:G] = zp[e, :, n0:n1]
            m[f"x{slot}"] = xa
            m[f"q{slot}"] = np.ascontiguousarray(qpm[e, :, :, n0:n1])
            m[f"s{slot}"] = np.ascontiguousarray(sc16[e, :, n0:n1])
            m[f"z{slot}"] = za
        bs = np.zeros([P // gs, P], BF16)
        for j in range(P // gs):
            bs[j, j * gs : (j + 1) * gs] = 1
        m["bsel"] = bs
        in_maps.append(m)

    res = run_bass_kernel_spmd(nc, in_maps, list(range(NCORES)))
    LAST_RESULT = res

    out = np.zeros([T, OUT], np.float32)
    for c in range(NCORES):
        for slot, (tt, e, h) in (("a", a_units[c]), ("b", b_units[c])):
            r0, r1 = int(bounds[e]), int(bounds[e + 1])
            out[r0:r1, h * NHALF : (h + 1) * NHALF] = res.results[c][f"y{slot}"][
                : r1 - r0
            ]
    return out
